# revision 11
# baseline (speedup 1.0000x reference)
"""Trainium2 Bass kernel for batched low-rank (rank-64) KV-cache reconstruction.

Problem: for each of 64 matrices X [2048,128] (f32), compute the rank-64
truncated-SVD reconstruction X_r = U_r diag(s_r) V_r^T = X P where P projects
onto the top-64 eigenspace of G = X^T X.

Per NeuronCore (8 matrices, two groups of 4), all-fp16 iterations:
  G = Xh^T Xh  (Xh = fp16(X), f32 PSUM accumulate)
  mu ladder: 4 probe stages of cubic soft-sign iterations (a=2 growth +
    a=1.5 landing steps) reading trace(B_k) as a soft eigen-count to
    root-find mu between lambda_64 and lambda_65 of G.
  final: 12-step sign iteration (optimized cubic coefficient schedule with
    periodic symmetric "pair" steps); P = (sign+I)/2 emitted by the last
    write-back.
  recon: Y^T = P @ Xh^T via fp16 matmuls (XT loaded by DMA-transpose XBAR).

The cubic step is computed as Ct = I - (b/a) B^2; B' = a * (B @ Ct), folding
`a` into the PSUM->SBUF write-back scale so one identity constant serves all
steps. Probes read trace(B@C) = sum(B*Ct)*a (B, Ct symmetric), skipping the
last matmul. The two groups run as interleaved instruction streams with a
skew of a few steps, so each group's serial DVE/ACT chain (Ct / write-back /
mu update) hides under the other group's PE matmuls.
"""

from contextlib import ExitStack

import numpy as np

import concourse.bass as bass
import concourse.tile as tile
from concourse import bacc, mybir
from concourse.bass_utils import run_bass_kernel_spmd
from concourse.masks import make_identity

F32 = mybir.dt.float32
F16 = mybir.dt.float16
AF = mybir.ActivationFunctionType
OP = mybir.AluOpType
ts = bass.ts

N_CORES = 8
M_PER_CORE = 8
NG = 2               # matrix groups per core
GM = 4               # matrices per group
S, D = 2048, 128
NT = S // 128        # 16 row chunks
SKEW = 3             # instruction-stream skew between the two groups (ticks)

R = 1250.0
OFF = -41.37

GROW = (2.0, 1.0)
LAND = (1.5, 0.5)
# ladder stages: (n_growth, n_land, gain, clamp)
LADDER = [
    (4, 1, 7.75, 28.0),
    (6, 1, 6.0, 8.0),
    (7, 2, 2.8, 4.0),
    (9, 2, 1.0, 1.2),
]
# final sign schedule (a, b), all fp16
FINAL = [
    (3.25, 3.10818),
    (2.59, 1.575008),
    (2.59, 1.571647),
    (2.59, 1.573557),
    (2.58, 1.553639),
    (2.56, 1.519041),
    (2.52, 1.450415),
    (2.4, 1.250828),
    (2.14, 0.887286),
    (1.427341, 0.407817),
    (1.500676, 0.500097),
]


def _pair_steps(k, every=4):
    """Every `every`-th step runs as a symmetric pair step (never the last)."""
    out = {i for i in range(k) if (i + 1) % every == 0}
    out.discard(k - 1)
    return out


class Ctx:
    pass


def _kernel_body(tc, nc, ctx, x, y):
    c = Ctx()
    consts = ctx.enter_context(tc.tile_pool(name="consts", bufs=1))
    c.ident = consts.tile([128, 128], F32, tag="ident", name="ident")
    make_identity(nc, c.ident[:])
    c.I_rep = consts.tile([128, 512], F32, tag="I_rep", name="I_rep")
    for b in range(GM):
        nc.gpsimd.tensor_copy(c.I_rep[:, ts(b, 128)], c.ident[:])
    c.I05_rep = consts.tile([128, 512], F32, tag="I05_rep", name="I05_rep")
    nc.vector.tensor_scalar_mul(c.I05_rep[:], c.I_rep[:], 0.5)
    c.ones_col = consts.tile([128, 1], F32, tag="ones_col", name="ones_col")
    nc.vector.memset(c.ones_col[:], 1.0)
    c.ones_row = consts.tile([1, 128], F32, tag="ones_row", name="ones_row")
    nc.vector.memset(c.ones_row[:], 1.0)
    c.neg_row = consts.tile([1, 128], F32, tag="neg_row", name="neg_row")
    nc.vector.memset(c.neg_row[:], -1.0)
    c.trpack = [consts.tile([128, GM], F32, tag=f"trpack{g}", name=f"trpack{g}") for g in range(NG)]
    c.mu_row = [consts.tile([1, GM], F32, tag=f"mu_row{g}", name=f"mu_row{g}") for g in range(NG)]
    c.step_row = [consts.tile([1, GM], F32, tag=f"step{g}", name=f"step{g}") for g in range(NG)]

    pools = Ctx()
    xt_pool = ctx.enter_context(tc.tile_pool(name="xt", bufs=1))
    xp_pool = ctx.enter_context(tc.tile_pool(name="xp", bufs=1))
    gpool = ctx.enter_context(tc.tile_pool(name="G", bufs=1))
    bpool = ctx.enter_context(tc.tile_pool(name="B", bufs=1))
    pools.ct = ctx.enter_context(tc.tile_pool(name="ct", bufs=2))
    pools.scr = ctx.enter_context(tc.tile_pool(name="scr", bufs=2))
    opool = ctx.enter_context(tc.tile_pool(name="osb", bufs=3))
    pools.p2 = ctx.enter_context(tc.tile_pool(name="p2", bufs=1, space="PSUM"))
    pools.p3 = ctx.enter_context(tc.tile_pool(name="p3", bufs=1, space="PSUM"))
    pools.pmisc = ctx.enter_context(tc.tile_pool(name="pmisc", bufs=1, space="PSUM"))

    XT = [xt_pool.tile([128, S], F16, tag=f"XT{m}", name=f"XT{m}") for m in range(M_PER_CORE)]
    XP = [xp_pool.tile([128, S], F16, tag=f"XP{m}", name=f"XP{m}", bufs=1)
          for m in range(M_PER_CORE)]
    G_all = [gpool.tile([128, 512], F32, tag=f"G{g}", name=f"G{g}") for g in range(NG)]
    B16 = [bpool.tile([128, 512], F16, tag=f"B16_{g}", name=f"B16_{g}") for g in range(NG)]
    P16 = [bpool.tile([128, 512], F16, tag=f"P16_{g}", name=f"P16_{g}") for g in range(NG)]

    def trace_cols(g, in0_ap, in1_ap):
        scr = pools.scr.tile([128, 512], F32, tag=f"scr{g}", name=f"scr{g}", bufs=2)
        nc.vector.tensor_tensor(scr[:], in0_ap, in1_ap, op=OP.mult)
        nc.vector.tensor_reduce(
            c.trpack[g][:, :], scr[:].rearrange("p (b d) -> p b d", d=128),
            axis=mybir.AxisListType.X, op=OP.add,
        )

    def mu_chain(g, scale, offset=None, clamp=None):
        """trpack -> mu_row update -> MU broadcast (one boundary tick)."""
        ps_tr = pools.pmisc.tile([1, GM], F32, tag=f"pstr{g}", name=f"pstr{g}", bufs=1)
        nc.tensor.matmul(ps_tr[:], c.ones_col[:], c.trpack[g][:], start=True, stop=True)
        if offset is not None:
            nc.vector.tensor_scalar(
                out=c.mu_row[g][:], in0=ps_tr[:], scalar1=scale, scalar2=offset,
                op0=OP.mult, op1=OP.add,
            )
        else:
            nc.vector.tensor_scalar(
                out=c.step_row[g][:], in0=ps_tr[:], scalar1=scale, scalar2=clamp,
                op0=OP.mult, op1=OP.min,
            )
            nc.vector.tensor_scalar_max(c.step_row[g][:], c.step_row[g][:], -clamp)
            nc.vector.tensor_tensor(
                c.mu_row[g][:], c.mu_row[g][:], c.step_row[g][:], op=OP.add
            )
        # broadcast NEGATED nu so b0_tick's STT can use a plain multiply
        ps_mu = pools.pmisc.tile([128, GM], F32, tag=f"psmu{g}", name=f"psmu{g}", bufs=1)
        nc.tensor.matmul(ps_mu[:], c.neg_row[:], c.mu_row[g][:], start=True, stop=True)
        return ps_mu

    def b0_tick(g, ps_mu):
        """B0 = G/R - nu*I per block: one STT per block, nu broadcast from PSUM."""
        for j in range(GM):
            nc.vector.scalar_tensor_tensor(
                out=B16[g][:, ts(j, 128)], in0=c.ident[:],
                scalar=ps_mu[:, j : j + 1], op0=OP.mult,
                in1=G_all[g][:, ts(j, 128)], op1=OP.add,
            )

    def cubic_step(g, a, b, is_pair, last_trace=False, emit_P=False):
        B = B16[g][:]
        ps2 = pools.p2.tile([128, 512], F32, tag=f"ps2g{g}", name=f"ps2g{g}", bufs=1)
        for j in range(GM):
            nc.tensor.matmul(
                ps2[:, ts(j, 128)], B[:, ts(j, 128)], B[:, ts(j, 128)],
                start=True, stop=True,
            )
        Ct = pools.ct.tile([128, 512], F16, tag=f"ct{g}", name=f"ct{g}", bufs=2)
        nc.vector.scalar_tensor_tensor(
            out=Ct[:], in0=ps2[:], scalar=-(b / a), in1=c.I_rep[:],
            op0=OP.mult, op1=OP.add,
        )
        if last_trace:
            trace_cols(g, B, Ct[:])
            return
        ps3 = pools.p3.tile([128, 512], F32, tag=f"ps3g{g}", name=f"ps3g{g}", bufs=1)
        for j in range(GM):
            nc.tensor.matmul(
                ps3[:, ts(j, 128)], B[:, ts(j, 128)], Ct[:, ts(j, 128)],
                start=True, stop=not is_pair,
            )
            if is_pair:
                # close each block's accumulation group before the next opens:
                # only one PSUM accumulation group may be open at a time
                nc.tensor.matmul(
                    ps3[:, ts(j, 128)], Ct[:, ts(j, 128)], B[:, ts(j, 128)],
                    start=False, stop=True, skip_group_check=True,
                )
        scale = a / 2.0 if is_pair else a
        if emit_P:
            nc.vector.scalar_tensor_tensor(
                out=P16[g][:], in0=ps3[:], scalar=0.5 * scale, in1=c.I05_rep[:],
                op0=OP.mult, op1=OP.add,
            )
        else:
            nc.scalar.activation(B, ps3[:], AF.Copy, scale=scale)

    def group_stream(g):
        """Yields once per 'tick'; emits that tick's instructions for group g."""
        # Gram + seed
        psG = pools.p2.tile([128, 512], F32, tag=f"ps2g{g}", name=f"psG{g}", bufs=1)
        for j in range(GM):
            m = g * GM + j
            for t in range(NT):
                nc.tensor.matmul(
                    psG[:, ts(j, 128)], XP[m][:, ts(t, 128)], XP[m][:, ts(t, 128)],
                    start=(t == 0), stop=(t == NT - 1),
                )
            yield
        nc.scalar.activation(G_all[g][:], psG[:], AF.Copy, scale=1.0 / R)
        trace_cols(g, G_all[g][:], c.I_rep[:])
        ps_mu = mu_chain(g, 1.0 / 128.0, offset=OFF / R)
        yield
        # ladder
        for (n_grow, n_land, gain, clamp) in LADDER:
            coeffs = [GROW] * n_grow + [LAND] * n_land
            k = len(coeffs)
            pair = _pair_steps(k)
            b0_tick(g, ps_mu)
            yield
            for i, (a, b) in enumerate(coeffs):
                cubic_step(g, a, b, i in pair, last_trace=(i == k - 1))
                yield
            a_last = coeffs[-1][0]
            ps_mu = mu_chain(g, gain * a_last / R, clamp=clamp / R)
            yield
        # final
        k = len(FINAL)
        # the final's free-coefficient growth steps amplify fp16 skew noise
        # (|f'(1)| ~ 2.1 per step), so symmetrize more often than the ladder
        pair = _pair_steps(k, every=3)
        b0_tick(g, ps_mu)
        yield
        for i, (a, b) in enumerate(FINAL):
            cubic_step(g, a, b, i in pair, emit_P=(i == k - 1))
            yield
        # recon
        for j in range(GM):
            m = g * GM + j
            for ch in range(S // 512):
                psO = pools.p3.tile([128, 512], F32, tag=f"ps3g{g}", name=f"psO{g}", bufs=1)
                nc.tensor.matmul(
                    psO[:], P16[g][:, ts(j, 128)], XT[m][:, ts(ch, 512)],
                    start=True, stop=True,
                )
                osb = opool.tile([128, 512], F16, tag=f"o{g}_{ch % 2}", name=f"o{g}", bufs=2)
                if ch % 2 == 0:
                    nc.scalar.copy(osb[:], psO[:])
                else:
                    nc.vector.tensor_copy(osb[:], psO[:])
                eng = nc.sync if (m + ch) % 2 == 0 else nc.scalar
                eng.dma_start(y[m, :, ts(ch, 512)], osb[:])
                yield

    # ---- input DMAs ----
    # packed row layout: XP[m][p, t*128+d] = X[m][p*16+t, d]  (4KB runs; any
    # row permutation is valid for the Gram accumulation)
    for m in range(M_PER_CORE):
        src = x[m].rearrange("(p t) d -> p t d", t=NT)
        dst = XP[m][:].rearrange("p (t d) -> p t d", d=128)
        eng = nc.sync if m % 2 == 0 else nc.scalar
        eng.dma_start(dst, src)
    # XT via DMA-transpose XBAR (needed only at recon; fills during ladder)
    for m in range(M_PER_CORE):
        nc.sync.dma_start(XT[m][:], x[m], transpose=True)

    # ---- interleave the two group streams with a skew ----
    streams = [group_stream(g) for g in range(NG)]
    done = [False] * NG
    tick = 0
    while not all(done):
        for g in (1, 0):
            if g == 1 and tick < SKEW:
                continue
            if not done[g]:
                try:
                    next(streams[g])
                except StopIteration:
                    done[g] = True
        tick += 1


_NC_CACHE = {}


def _build_program():
    if "nc" in _NC_CACHE:
        return _NC_CACHE["nc"]
    nc = bacc.Bacc(
        "TRN2",
        target_bir_lowering=False,
        debug=False,
        enable_asserts=True,
        num_devices=N_CORES,
    )
    x = nc.dram_tensor("x", [M_PER_CORE, S, D], F16, kind="ExternalInput").ap()
    y = nc.dram_tensor("y", [M_PER_CORE, D, S], F16, kind="ExternalOutput").ap()
    with tile.TileContext(nc) as tc:
        with ExitStack() as ctx:
            _kernel_body(tc, nc, ctx, x, y)
    nc.compile()
    _NC_CACHE["nc"] = nc
    return nc


def kernel(kv_cache, rank, **_ignored):
    kv = np.asarray(kv_cache)
    assert kv.shape == (4, 16, S, D), kv.shape
    assert int(rank) == 64, rank
    orig_dtype = kv.dtype
    xs = np.ascontiguousarray(kv.reshape(-1, S, D)).astype(np.float16)

    nc = _build_program()
    in_maps = [
        {"x": xs[i * M_PER_CORE : (i + 1) * M_PER_CORE]} for i in range(N_CORES)
    ]
    res = run_bass_kernel_spmd(nc, in_maps, list(range(N_CORES)))
    outs = [np.asarray(res.results[i]["y"]) for i in range(N_CORES)]
    yt = np.concatenate(outs, axis=0)          # [64, 128, 2048] f16
    out = yt.astype(np.float32).transpose(0, 2, 1).reshape(4, 16, S, D)
    return out.astype(orig_dtype, copy=False)


if __name__ == "__main__":
    rng = np.random.default_rng(0)
    kv = rng.standard_normal((4, 16, S, D)).astype(np.float32)
    out = kernel(kv_cache=kv, rank=64)
    print("kernel ran, out", out.shape, out.dtype)


# revision 14
# speedup vs baseline: 1.0112x; 1.0112x over previous
"""Trainium2 Bass kernel for batched low-rank (rank-64) KV-cache reconstruction.

Problem: for each of 64 matrices X [2048,128] (f32), compute the rank-64
truncated-SVD reconstruction X_r = U_r diag(s_r) V_r^T = X P where P projects
onto the top-64 eigenspace of G = X^T X.

Per NeuronCore (8 matrices, two groups of 4), all-fp16 iterations:
  G = Xh^T Xh  (Xh = fp16(X), f32 PSUM accumulate)
  mu ladder: 4 probe stages of cubic soft-sign iterations (a=2 growth +
    a=1.5 landing steps) reading trace(B_k) as a soft eigen-count to
    root-find mu between lambda_64 and lambda_65 of G.
  final: 12-step sign iteration (optimized cubic coefficient schedule with
    periodic symmetric "pair" steps); P = (sign+I)/2 emitted by the last
    write-back.
  recon: Y^T = P @ Xh^T via fp16 matmuls (XT loaded by DMA-transpose XBAR).

The cubic step is computed as Ct = I - (b/a) B^2; B' = a * (B @ Ct), folding
`a` into the PSUM->SBUF write-back scale so one identity constant serves all
steps. Probes read trace(B@C) = sum(B*Ct)*a (B, Ct symmetric), skipping the
last matmul. The two groups run as interleaved instruction streams with a
skew of a few steps, so each group's serial DVE/ACT chain (Ct / write-back /
mu update) hides under the other group's PE matmuls.
"""

from contextlib import ExitStack

import numpy as np

import concourse.bass as bass
import concourse.tile as tile
from concourse import bacc, mybir
from concourse.bass_utils import run_bass_kernel_spmd
from concourse.masks import make_identity

F32 = mybir.dt.float32
F16 = mybir.dt.float16
AF = mybir.ActivationFunctionType
OP = mybir.AluOpType
ts = bass.ts

N_CORES = 8
M_PER_CORE = 8
NG = 2               # matrix groups per core
GM = 4               # matrices per group
S, D = 2048, 128
NT = S // 128        # 16 row chunks
SKEW = 4             # instruction-stream skew between the two groups (ticks)

R = 1250.0
OFF = -41.37

GROW = (2.0, 1.0)
LAND = (1.5, 0.5)
# ladder stages: (n_growth, n_land, gain, clamp)
LADDER = [
    (3, 1, 7.75, 28.0),
    (6, 1, 6.0, 8.0),
    (7, 2, 2.8, 4.0),
    (9, 2, 1.0, 1.2),
]
# final sign schedule (a, b), all fp16
FINAL = [
    (3.25, 3.10818),
    (2.59, 1.575008),
    (2.59, 1.571647),
    (2.59, 1.573557),
    (2.58, 1.553639),
    (2.56, 1.519041),
    (2.52, 1.450415),
    (2.4, 1.250828),
    (2.14, 0.887286),
    (1.427341, 0.407817),
    (1.500676, 0.500097),
]


def _pair_steps(k, every=4):
    """Every `every`-th step runs as a symmetric pair step (never the last)."""
    out = {i for i in range(k) if (i + 1) % every == 0}
    out.discard(k - 1)
    return out


class Ctx:
    pass


def _kernel_body(tc, nc, ctx, x, y):
    c = Ctx()
    consts = ctx.enter_context(tc.tile_pool(name="consts", bufs=1))
    c.ident = consts.tile([128, 128], F32, tag="ident", name="ident")
    make_identity(nc, c.ident[:])
    c.I_rep = consts.tile([128, 512], F32, tag="I_rep", name="I_rep")
    for b in range(GM):
        nc.gpsimd.tensor_copy(c.I_rep[:, ts(b, 128)], c.ident[:])
    c.I05_rep = consts.tile([128, 512], F32, tag="I05_rep", name="I05_rep")
    nc.vector.tensor_scalar_mul(c.I05_rep[:], c.I_rep[:], 0.5)
    c.ones_col = consts.tile([128, 1], F32, tag="ones_col", name="ones_col")
    nc.vector.memset(c.ones_col[:], 1.0)
    c.ones_row = consts.tile([1, 128], F32, tag="ones_row", name="ones_row")
    nc.vector.memset(c.ones_row[:], 1.0)
    c.neg_row = consts.tile([1, 128], F32, tag="neg_row", name="neg_row")
    nc.vector.memset(c.neg_row[:], -1.0)
    c.trpack = [consts.tile([128, GM], F32, tag=f"trpack{g}", name=f"trpack{g}") for g in range(NG)]
    c.mu_row = [consts.tile([1, GM], F32, tag=f"mu_row{g}", name=f"mu_row{g}") for g in range(NG)]
    c.step_row = [consts.tile([1, GM], F32, tag=f"step{g}", name=f"step{g}") for g in range(NG)]

    pools = Ctx()
    xt_pool = ctx.enter_context(tc.tile_pool(name="xt", bufs=1))
    xp_pool = ctx.enter_context(tc.tile_pool(name="xp", bufs=1))
    gpool = ctx.enter_context(tc.tile_pool(name="G", bufs=1))
    bpool = ctx.enter_context(tc.tile_pool(name="B", bufs=1))
    pools.ct = ctx.enter_context(tc.tile_pool(name="ct", bufs=2))
    pools.scr = ctx.enter_context(tc.tile_pool(name="scr", bufs=2))
    opool = ctx.enter_context(tc.tile_pool(name="osb", bufs=3))
    pools.p2 = ctx.enter_context(tc.tile_pool(name="p2", bufs=1, space="PSUM"))
    pools.p3 = ctx.enter_context(tc.tile_pool(name="p3", bufs=1, space="PSUM"))
    pools.pmisc = ctx.enter_context(tc.tile_pool(name="pmisc", bufs=1, space="PSUM"))

    XT = [xt_pool.tile([128, S], F16, tag=f"XT{m}", name=f"XT{m}") for m in range(M_PER_CORE)]
    XP = [xp_pool.tile([128, S], F16, tag=f"XP{m}", name=f"XP{m}", bufs=1)
          for m in range(M_PER_CORE)]
    # ---- input DMAs first: transfers overlap constant building ----
    # packed row layout: XP[m][p, t*128+d] = X[m][p*16+t, d]  (4KB runs; any
    # row permutation is valid for the Gram accumulation)
    for m in range(M_PER_CORE):
        src_ap = x[m].rearrange("(p t) d -> p t d", t=NT)
        dst_ap = XP[m][:].rearrange("p (t d) -> p t d", d=128)
        eng = nc.sync if m % 2 == 0 else nc.scalar
        eng.dma_start(dst_ap, src_ap)
    # XT via DMA-transpose XBAR (needed only at recon; fills during ladder)
    for m in range(M_PER_CORE):
        nc.sync.dma_start(XT[m][:], x[m], transpose=True)

    G_all = [gpool.tile([128, 512], F32, tag=f"G{g}", name=f"G{g}") for g in range(NG)]
    B16 = [bpool.tile([128, 512], F16, tag=f"B16_{g}", name=f"B16_{g}") for g in range(NG)]
    P16 = [bpool.tile([128, 512], F16, tag=f"P16_{g}", name=f"P16_{g}") for g in range(NG)]

    def trace_cols(g, in0_ap, in1_ap):
        scr = pools.scr.tile([128, 512], F32, tag=f"scr{g}", name=f"scr{g}", bufs=2)
        nc.vector.tensor_tensor(scr[:], in0_ap, in1_ap, op=OP.mult)
        nc.vector.tensor_reduce(
            c.trpack[g][:, :], scr[:].rearrange("p (b d) -> p b d", d=128),
            axis=mybir.AxisListType.X, op=OP.add,
        )

    def mu_chain(g, scale, offset=None, clamp=None):
        """trpack -> mu_row update -> MU broadcast (one boundary tick)."""
        ps_tr = pools.pmisc.tile([1, GM], F32, tag=f"pstr{g}", name=f"pstr{g}", bufs=1)
        nc.tensor.matmul(ps_tr[:], c.ones_col[:], c.trpack[g][:], start=True, stop=True)
        if offset is not None:
            nc.vector.tensor_scalar(
                out=c.mu_row[g][:], in0=ps_tr[:], scalar1=scale, scalar2=offset,
                op0=OP.mult, op1=OP.add,
            )
        else:
            nc.vector.tensor_scalar(
                out=c.step_row[g][:], in0=ps_tr[:], scalar1=scale, scalar2=clamp,
                op0=OP.mult, op1=OP.min,
            )
            nc.vector.tensor_scalar_max(c.step_row[g][:], c.step_row[g][:], -clamp)
            nc.vector.tensor_tensor(
                c.mu_row[g][:], c.mu_row[g][:], c.step_row[g][:], op=OP.add
            )
        # broadcast NEGATED nu so b0_tick's STT can use a plain multiply
        ps_mu = pools.pmisc.tile([128, GM], F32, tag=f"psmu{g}", name=f"psmu{g}", bufs=1)
        nc.tensor.matmul(ps_mu[:], c.neg_row[:], c.mu_row[g][:], start=True, stop=True)
        return ps_mu

    def b0_tick(g, ps_mu):
        """B0 = G/R - nu*I per block: one STT per block, nu broadcast from PSUM."""
        for j in range(GM):
            nc.vector.scalar_tensor_tensor(
                out=B16[g][:, ts(j, 128)], in0=c.ident[:],
                scalar=ps_mu[:, j : j + 1], op0=OP.mult,
                in1=G_all[g][:, ts(j, 128)], op1=OP.add,
            )

    def cubic_step(g, a, b, is_pair, last_trace=False, emit_P=False):
        B = B16[g][:]
        ps2 = pools.p2.tile([128, 512], F32, tag=f"ps2g{g}", name=f"ps2g{g}", bufs=1)
        for j in range(GM):
            nc.tensor.matmul(
                ps2[:, ts(j, 128)], B[:, ts(j, 128)], B[:, ts(j, 128)],
                start=True, stop=True,
            )
        Ct = pools.ct.tile([128, 512], F16, tag=f"ct{g}", name=f"ct{g}", bufs=2)
        nc.vector.scalar_tensor_tensor(
            out=Ct[:], in0=ps2[:], scalar=-(b / a), in1=c.I_rep[:],
            op0=OP.mult, op1=OP.add,
        )
        if last_trace:
            trace_cols(g, B, Ct[:])
            return
        ps3 = pools.p3.tile([128, 512], F32, tag=f"ps3g{g}", name=f"ps3g{g}", bufs=1)
        for j in range(GM):
            nc.tensor.matmul(
                ps3[:, ts(j, 128)], B[:, ts(j, 128)], Ct[:, ts(j, 128)],
                start=True, stop=not is_pair,
            )
            if is_pair:
                # close each block's accumulation group before the next opens:
                # only one PSUM accumulation group may be open at a time
                nc.tensor.matmul(
                    ps3[:, ts(j, 128)], Ct[:, ts(j, 128)], B[:, ts(j, 128)],
                    start=False, stop=True, skip_group_check=True,
                )
        scale = a / 2.0 if is_pair else a
        if emit_P:
            # per-block so each matrix's recon can start as soon as its block lands
            for j in range(GM):
                nc.vector.scalar_tensor_tensor(
                    out=P16[g][:, ts(j, 128)], in0=ps3[:, ts(j, 128)],
                    scalar=0.5 * scale, in1=c.I05_rep[:, ts(j, 128)],
                    op0=OP.mult, op1=OP.add,
                )
        else:
            nc.scalar.activation(B, ps3[:], AF.Copy, scale=scale)

    def group_stream(g):
        """Yields once per 'tick'; emits that tick's instructions for group g."""
        # Gram + seed
        psG = pools.p2.tile([128, 512], F32, tag=f"ps2g{g}", name=f"psG{g}", bufs=1)
        for j in range(GM):
            m = g * GM + j
            for t in range(NT):
                nc.tensor.matmul(
                    psG[:, ts(j, 128)], XP[m][:, ts(t, 128)], XP[m][:, ts(t, 128)],
                    start=(t == 0), stop=(t == NT - 1),
                )
            yield
        nc.scalar.activation(G_all[g][:], psG[:], AF.Copy, scale=1.0 / R)
        trace_cols(g, G_all[g][:], c.I_rep[:])
        ps_mu = mu_chain(g, 1.0 / 128.0, offset=OFF / R)
        yield
        # ladder
        for (n_grow, n_land, gain, clamp) in LADDER:
            coeffs = [GROW] * n_grow + [LAND] * n_land
            k = len(coeffs)
            pair = _pair_steps(k)
            b0_tick(g, ps_mu)
            yield
            for i, (a, b) in enumerate(coeffs):
                cubic_step(g, a, b, i in pair, last_trace=(i == k - 1))
                yield
            a_last = coeffs[-1][0]
            ps_mu = mu_chain(g, gain * a_last / R, clamp=clamp / R)
            yield
        # final
        k = len(FINAL)
        # the final's free-coefficient growth steps amplify fp16 skew noise
        # (|f'(1)| ~ 2.1 per step), so symmetrize more often than the ladder
        pair = _pair_steps(k, every=3)
        b0_tick(g, ps_mu)
        yield
        for i, (a, b) in enumerate(FINAL):
            cubic_step(g, a, b, i in pair, emit_P=(i == k - 1))
            yield
        # recon
        for j in range(GM):
            m = g * GM + j
            for ch in range(S // 512):
                psO = pools.p3.tile([128, 512], F32, tag=f"ps3g{g}", name=f"psO{g}", bufs=1)
                nc.tensor.matmul(
                    psO[:], P16[g][:, ts(j, 128)], XT[m][:, ts(ch, 512)],
                    start=True, stop=True,
                )
                osb = opool.tile([128, 512], F16, tag=f"o{g}_{ch % 2}", name=f"o{g}", bufs=2)
                if ch % 2 == 0:
                    nc.scalar.copy(osb[:], psO[:])
                else:
                    nc.vector.tensor_copy(osb[:], psO[:])
                eng = nc.sync if (m + ch) % 2 == 0 else nc.scalar
                eng.dma_start(y[m, :, ts(ch, 512)], osb[:])
                yield

    # ---- interleave the two group streams with a skew ----
    streams = [group_stream(g) for g in range(NG)]
    done = [False] * NG
    tick = 0
    while not all(done):
        for g in (1, 0):
            if g == 1 and tick < SKEW:
                continue
            if not done[g]:
                try:
                    next(streams[g])
                except StopIteration:
                    done[g] = True
        tick += 1


_NC_CACHE = {}


def _build_program():
    if "nc" in _NC_CACHE:
        return _NC_CACHE["nc"]
    nc = bacc.Bacc(
        "TRN2",
        target_bir_lowering=False,
        debug=False,
        enable_asserts=True,
        num_devices=N_CORES,
    )
    x = nc.dram_tensor("x", [M_PER_CORE, S, D], F16, kind="ExternalInput").ap()
    y = nc.dram_tensor("y", [M_PER_CORE, D, S], F16, kind="ExternalOutput").ap()
    with tile.TileContext(nc) as tc:
        with ExitStack() as ctx:
            _kernel_body(tc, nc, ctx, x, y)
    nc.compile()
    _NC_CACHE["nc"] = nc
    return nc


def kernel(kv_cache, rank, **_ignored):
    kv = np.asarray(kv_cache)
    assert kv.shape == (4, 16, S, D), kv.shape
    assert int(rank) == 64, rank
    orig_dtype = kv.dtype
    xs = np.ascontiguousarray(kv.reshape(-1, S, D)).astype(np.float16)

    nc = _build_program()
    in_maps = [
        {"x": xs[i * M_PER_CORE : (i + 1) * M_PER_CORE]} for i in range(N_CORES)
    ]
    res = run_bass_kernel_spmd(nc, in_maps, list(range(N_CORES)))
    outs = [np.asarray(res.results[i]["y"]) for i in range(N_CORES)]
    yt = np.concatenate(outs, axis=0)          # [64, 128, 2048] f16
    out = yt.astype(np.float32).transpose(0, 2, 1).reshape(4, 16, S, D)
    return out.astype(orig_dtype, copy=False)


if __name__ == "__main__":
    rng = np.random.default_rng(0)
    kv = rng.standard_normal((4, 16, S, D)).astype(np.float32)
    out = kernel(kv_cache=kv, rank=64)
    print("kernel ran, out", out.shape, out.dtype)
